# revision 1
# baseline (speedup 1.0000x reference)
"""Distributed 2-layer GCN kernel for 8 TRN2 NeuronCores (dev version:
imports sibling modules; will be inlined for submission)."""

import sys

sys.path.insert(0, "/root/problem")

import numpy as np

import gcn_host
import gcn_bass

T_HALF = 7
SB_BLOCKS = 4
N_CORES = 8

LAST_RUN_INFO = {}


def kernel(x, edge_index, edge_weight, W1, b1, W2, b2):
    from concourse.bass_utils import run_bass_kernel_spmd

    x = np.asarray(x, dtype=np.float32)
    W1 = np.asarray(W1, dtype=np.float32)
    W2 = np.asarray(W2, dtype=np.float32)
    b1 = np.asarray(b1, dtype=np.float32)
    b2 = np.asarray(b2, dtype=np.float32)
    N, hid = x.shape
    out_dim = W2.shape[1]

    plan = gcn_host.build_plan(edge_index, edge_weight, N, N_CORES,
                               t_half=T_HALF, sb_blocks=SB_BLOCKS)
    has_b1 = bool(np.any(b1 != 0))
    has_b2 = bool(np.any(b2 != 0))
    nc = gcn_bass.build_gcn_nc(plan, has_b1, has_b2, hid, out_dim)

    x16 = x.astype(np.float16)
    w1_16 = W1.astype(np.float16)
    w2_16 = W2.astype(np.float16)
    in_maps = [
        gcn_bass.make_in_map(plan, c, x16, w1_16, w2_16, b1, b2,
                             has_b1, has_b2)
        for c in range(N_CORES)
    ]

    trace = bool(int(os.environ.get("GCN_TRACE", "0"))) if True else False
    res = run_bass_kernel_spmd(nc, in_maps, core_ids=list(range(N_CORES)),
                               trace=trace)
    LAST_RUN_INFO.clear()
    LAST_RUN_INFO["exec_time_ns"] = res.exec_time_ns
    if res.instructions_and_trace is not None:
        LAST_RUN_INFO["trace_path"] = res.instructions_and_trace[1]

    out = gcn_host.unpack_output(plan, res.results, "out_pad", out_dim)
    return out


import os  # noqa: E402
